# revision 3
# baseline (speedup 1.0000x reference)
"""Trainium2 Bass kernel v2 for CharacterBERT CharCNN.

v2 design:
  - Embedding gather AND skew done on HOST: device receives a ready
    x_skew=[120, 25600] bf16 (112 shifted-embedding rows + 7 mask rows +
    ones row), uploaded in column slices so conv starts almost immediately.
  - Conv = one matmul per 128-filter chunk against combined width-padded
    weights W_all [120, 2048] (masks inject -30000; bias via ones row).
  - Maxpool+relu via split ACT/DVE pooling: ACT-path chunks are drained
    PSUM->SBUF bf16 by the scalar engine (free Relu) in pos-major layout,
    then reduced by a 6-level DVE tensor_tensor max tree (2x mode);
    DVE-path chunks use one 1x tensor_reduce per PSUM tile.  ACT-path and
    DVE-path chunks are processed in PAIRS with tile-level interleaving and
    a 3-slot PSUM ring so both drain engines run concurrently.
  - Highway/proj matmuls per 256-token half; half-0 highway interleaves
    with half-1 conv+drain in PE program order (PE executes in-order).
"""

import numpy as np
import ml_dtypes
from contextlib import ExitStack

import concourse.bass as bass
import concourse.mybir as mybir
import concourse.tile as tile
from concourse import bacc
from concourse.bass_utils import run_bass_kernel_spmd

BF16 = mybir.dt.bfloat16
F32 = mybir.dt.float32
AF = mybir.ActivationFunctionType
ALU = mybir.AluOpType

# problem geometry (hardcoded)
B, S, MAX_CHARS = 8, 512, 50
EMB = 16
VOCAB = 264
TOTAL_F = 2048
HIDDEN = 768
FILTERS = [(1, 32), (2, 32), (3, 64), (4, 128), (5, 256), (6, 512), (7, 1024)]
NCORES = 8

T = 512                 # tokens per core
HALF = 256
P50 = MAX_CHARS
COLS = T * P50          # 25600
KCONV = 120             # 112 patch rows + 7 mask rows + 1 bias row
NEG = -30000.0
NKF = TOTAL_F // 128    # 16 filter chunks
NKH = TOTAL_F // 128    # 16 contraction chunks
NOP = HIDDEN // 128     # 6 proj chunks
NSLICE = 8              # x_skew upload column slices
WS = 64.0               # fp8 highway-weight scale

# valid conv output positions per 128-filter chunk (width-trimmed):
# chunk 0 = w1+w2+w3 (needs 50, masked), 1 = w4, 2-3 = w5, 4-7 = w6, 8-15 = w7
PCOL = [50, 47, 46, 46, 45, 45, 45, 45] + [44] * 8

# chunk processing order per half: (ACT-path chunk, DVE-path chunk or None)
PAIR_ORDER = [(0, 2), (1, None), (3, 5), (4, None), (6, 8), (7, None),
              (9, 11), (10, None), (12, 14), (13, None), (15, None)]
DVE_CHUNKS = frozenset(d for _, d in PAIR_ORDER if d is not None)

_BF = ml_dtypes.bfloat16


def _bf(x):
    return np.asarray(x, dtype=np.float32).astype(_BF)


def _tiles_of(n, step):
    out, t0 = [], 0
    while t0 < n:
        out.append((t0, min(step, n - t0)))
        t0 += step
    return out


def _tree_plan(p):
    """Overlapped pairwise-max splits: P -> a=ceil(P/2) via max(x[0:a], x[P-a:P])."""
    plan = []
    while p > 1:
        a = (p + 1) // 2
        plan.append((p, a))
        p = a
    return plan


def build_program(ExitStackCls=ExitStack, loop_n=1):
    nc = bacc.Bacc("TRN2", target_bir_lowering=False, debug=False)

    F8 = mybir.dt.float8e4
    d_XS = nc.dram_tensor("XS", [KCONV, COLS], BF16, kind="ExternalInput").ap()
    d_wall = nc.dram_tensor("wall", [KCONV, TOTAL_F], BF16, kind="ExternalInput").ap()
    d_hw0w = nc.dram_tensor("hw0w", [32, 128, TOTAL_F], F8, kind="ExternalInput").ap()
    d_hw1w = nc.dram_tensor("hw1w", [32, 128, TOTAL_F], F8, kind="ExternalInput").ap()
    d_prjw = nc.dram_tensor("prjw", [NOP, 128, TOTAL_F], BF16, kind="ExternalInput").ap()
    d_hwb = nc.dram_tensor("hwb", [128, 64], F32, kind="ExternalInput").ap()
    d_prjb = nc.dram_tensor("prjb", [128, NOP], F32, kind="ExternalInput").ap()
    d_out = nc.dram_tensor("out", [NOP, 128, T], F32, kind="ExternalOutput").ap()
    d_hw_w = [d_hw0w, d_hw1w]

    with tile.TileContext(nc) as tc, ExitStackCls() as ctx:
        const = ctx.enter_context(tc.tile_pool(name="const", bufs=1))
        xsk_p = ctx.enter_context(tc.tile_pool(name="xsk", bufs=1))
        dr_p = ctx.enter_context(tc.tile_pool(name="dr", bufs=2))
        t_p = ctx.enter_context(tc.tile_pool(name="tmaj", bufs=1))
        hww_p = ctx.enter_context(tc.tile_pool(name="hww", bufs=3))
        hwt_p = ctx.enter_context(tc.tile_pool(name="hwt", bufs=2))
        out_p = ctx.enter_context(tc.tile_pool(name="outp", bufs=2))
        convps = ctx.enter_context(tc.tile_pool(name="convps", bufs=3, space="PSUM"))
        hwps = ctx.enter_context(tc.tile_pool(name="hwps", bufs=2, space="PSUM"))

        # ---- constants + skewed input (host-built, column-sliced upload) ----
        wall_t = const.tile([KCONV, TOTAL_F], BF16)
        nc.sync.dma_start(wall_t[:], d_wall[:])
        hwb_t = const.tile([128, 64], F32)
        nc.sync.dma_start(hwb_t[:], d_hwb[:])
        prjb_t = const.tile([128, NOP], F32)
        nc.sync.dma_start(prjb_t[:], d_prjb[:])

        x_skew = xsk_p.tile([KCONV, COLS], BF16)
        for (s0, ns) in _tiles_of(COLS, COLS // NSLICE):
            nc.sync.dma_start(out=x_skew[:, s0:s0 + ns],
                              in_=d_XS[:, s0:s0 + ns])

        # per-half t tiles, 3 generations (t0 drained, t1, t2)
        t_tiles = {}
        for gen in range(3):
            for h in range(2):
                t_tiles[(gen, h)] = t_p.tile([128, NKF, HALF], BF16,
                                             tag=f"t{gen}{h}", name=f"t{gen}{h}")

        loop_cm = tc.For_i(0, loop_n) if loop_n > 1 else None
        if loop_cm is not None:
            loop_cm.__enter__()

        x3 = x_skew[:].rearrange("p (t q) -> p t q", q=P50)

        def tok_groups(ck):
            """Drain-tile token groups per half for chunk ck.

            Each tile holds up to 2 matmul groups of n=floor(512/P) tokens,
            each MM landing in its own PSUM bank (offsets 0 and 512)."""
            p = PCOL[ck]
            n = 512 // p
            tiles, t0 = [], 0
            while t0 < HALF:
                g1 = min(n, HALF - t0)
                g2 = min(n, HALF - t0 - g1)
                tiles.append((t0, g1, g2))
                t0 += g1 + g2
            return tiles

        def conv_tile(h, ck, tt0, g1, g2):
            """Conv matmuls for one drain tile; returns the psum tile."""
            p = PCOL[ck]
            ps = convps.tile([128, 1024], F32, tag="cps", name="cps")
            base_t = h * HALF + tt0
            for k, ng in enumerate((g1, g2)):
                if ng:
                    nc.tensor.matmul(
                        ps[:, 512 * k:512 * k + ng * p],
                        lhsT=wall_t[:, 128 * ck:128 * (ck + 1)],
                        rhs=x3[:, base_t:base_t + ng, 0:p] if k == 0 else
                            x3[:, base_t + g1:base_t + g1 + ng, 0:p],
                        start=True, stop=True,
                    )
            return ps

        def drain_act(ps, dr, ck, tt0, g1, g2):
            p = PCOL[ck]
            if g1 == g2:
                nc.scalar.activation(
                    out=dr[:, 0:p, tt0:tt0 + 2 * g1].transpose([0, 2, 1]),
                    in_=ps[:].rearrange("p (s c) -> p s c", s=2)[:, :, 0:g1 * p],
                    func=AF.Relu,
                )
            else:
                for k, (o, ng) in enumerate(((0, g1), (g1, g2))):
                    if ng:
                        nc.scalar.activation(
                            out=dr[:, 0:p, tt0 + o:tt0 + o + ng]
                                .transpose([0, 2, 1]),
                            in_=ps[:, 512 * k:512 * k + ng * p],
                            func=AF.Relu,
                        )

        def drain_dve(ps, t0t, ck, tt0, g1, g2):
            p = PCOL[ck]
            if g1 == g2:
                nc.vector.tensor_reduce(
                    out=t0t[:, ck, tt0:tt0 + 2 * g1],
                    in_=ps[:].rearrange("p (s c) -> p s c", s=2)
                        [:, :, 0:g1 * p].rearrange("p s (t q) -> p s t q", q=p),
                    axis=mybir.AxisListType.X,
                    op=ALU.max,
                )
            else:
                for k, (o, ng) in enumerate(((0, g1), (g1, g2))):
                    if ng:
                        nc.vector.tensor_reduce(
                            out=t0t[:, ck, tt0 + o:tt0 + o + ng],
                            in_=ps[:, 512 * k:512 * k + ng * p]
                                .rearrange("p (t q) -> p t q", q=p),
                            axis=mybir.AxisListType.X,
                            op=ALU.max,
                        )

        def tree(dr, t0t, ck):
            # pairwise-max tree; ping-pong row regions A=[50,75) B=[0,50)
            dst_seq = [50, 0, 25, 0, 25]
            cur_base = 0
            plan = _tree_plan(PCOL[ck])
            for li, (p, a) in enumerate(plan):
                in0 = dr[:, cur_base:cur_base + a, :]
                in1 = dr[:, cur_base + p - a:cur_base + p, :]
                if li == len(plan) - 1:
                    nc.vector.tensor_max(t0t[:, ck, :], in0[:, 0, :],
                                         in1[:, 0, :])
                else:
                    dst = dst_seq[li]
                    nc.vector.tensor_max(dr[:, dst:dst + a, :], in0, in1)
                    cur_base = dst

        def conv_drain_group(h, ack, dck):
            """One ACT-path chunk (+ optionally one DVE-path chunk),
            tile-interleaved so ACT and DVE drain concurrently."""
            t0t = t_tiles[(0, h)]
            dr = dr_p.tile([128, 75, HALF], BF16, tag="dr", name="dr")
            ta = tok_groups(ack)
            td = tok_groups(dck) if dck is not None else []
            for i in range(max(len(ta), len(td))):
                ps_a = conv_tile(h, ack, *ta[i]) if i < len(ta) else None
                ps_d = conv_tile(h, dck, *td[i]) if i < len(td) else None
                if ps_a is not None:
                    drain_act(ps_a, dr, ack, *ta[i])
                if ps_d is not None:
                    drain_dve(ps_d, t0t, dck, *td[i])
            tree(dr, t0t, ack)
            if dck is not None:
                nc.vector.tensor_scalar_max(t0t[:, dck, :], t0t[:, dck, :], 0.0)

        def hw_j_group(layer, h, j):
            t_in = t_tiles[(layer, h)]
            t_out = t_tiles[(layer + 1, h)]
            w2 = hww_p.tile([128, 2 * TOTAL_F], F8, tag="w", name="w2")
            nc.sync.dma_start(w2[:, 0:TOTAL_F], d_hw_w[layer][j, :, :])
            nc.sync.dma_start(w2[:, TOTAL_F:2 * TOTAL_F],
                              d_hw_w[layer][j + 16, :, :])
            b_nl = hwb_t[:, layer * 32 + j:layer * 32 + j + 1]
            b_g = hwb_t[:, layer * 32 + 16 + j:layer * 32 + 16 + j + 1]
            ps = hwps.tile([128, 512], F32, tag="hw", name="hwps")
            for kc in range(NKH):
                nc.tensor.matmul(
                    ps[:, 0:HALF],
                    lhsT=w2[:, 128 * kc:128 * (kc + 1)],
                    rhs=t_in[:, kc, :],
                    start=(kc == 0), stop=(kc == NKH - 1),
                )
            for kc in range(NKH):
                nc.tensor.matmul(
                    ps[:, HALF:2 * HALF],
                    lhsT=w2[:, TOTAL_F + 128 * kc:TOTAL_F + 128 * (kc + 1)],
                    rhs=t_in[:, kc, :],
                    start=(kc == 0), stop=(kc == NKH - 1),
                )
            sg = hwt_p.tile([128, HALF], BF16, tag="sg", name="sg")
            nc.scalar.activation(sg[:], ps[:, HALF:2 * HALF], AF.Sigmoid,
                                 bias=b_g, scale=1.0 / WS)
            rl = hwt_p.tile([128, HALF], BF16, tag="rl", name="rl")
            nc.scalar.activation(rl[:], ps[:, 0:HALF], AF.Relu, bias=b_nl,
                                 scale=1.0 / WS)
            dd = hwt_p.tile([128, HALF], BF16, tag="dd", name="dd")
            nc.vector.tensor_sub(dd[:], t_in[:, j, :], rl[:])
            ee = hwt_p.tile([128, HALF], BF16, tag="ee", name="ee")
            nc.vector.tensor_mul(ee[:], sg[:], dd[:])
            nc.vector.tensor_add(t_out[:, j, :], ee[:], rl[:])

        def proj_half(h):
            t_in = t_tiles[(2, h)]
            for o in range(NOP):
                w2 = hww_p.tile([128, 2 * TOTAL_F], BF16, tag="w", name="w2")
                nc.sync.dma_start(w2[:, 0:TOTAL_F], d_prjw[o, :, :])
                ps = hwps.tile([128, 512], F32, tag="hw", name="hwps")
                for kc in range(NKH):
                    nc.tensor.matmul(
                        ps[:, 0:HALF],
                        lhsT=w2[:, 128 * kc:128 * (kc + 1)],
                        rhs=t_in[:, kc, :],
                        start=(kc == 0), stop=(kc == NKH - 1),
                    )
                ot = out_p.tile([128, HALF], F32, tag="ot", name="ot")
                nc.scalar.activation(ot[:], ps[:, 0:HALF], AF.Identity,
                                     bias=prjb_t[:, o:o + 1])
                nc.sync.dma_start(out=d_out[o, :, h * HALF:(h + 1) * HALF],
                                  in_=ot[:])

        # ---- emission order (PE in-order; this shapes the overlap) ----
        for (ack, dck) in PAIR_ORDER:
            conv_drain_group(0, ack, dck)

        # interleave conv+drain(half 1) with highway L1+L2(half 0):
        # 11 conv groups vs 32 hw j-groups -> conv, hw, hw, hw, conv, ...
        hw0_units = [(0, j) for j in range(NKH)] + [(1, j) for j in range(NKH)]
        ci = hi = 0
        while ci < len(PAIR_ORDER) or hi < len(hw0_units):
            if ci < len(PAIR_ORDER) and (hi >= len(hw0_units)
                                         or hi * len(PAIR_ORDER) >= ci * len(hw0_units)):
                ack, dck = PAIR_ORDER[ci]
                conv_drain_group(1, ack, dck)
                ci += 1
            else:
                layer, j = hw0_units[hi]
                hw_j_group(layer, 0, j)
                hi += 1

        for j in range(NKH):
            hw_j_group(0, 1, j)
        proj_half(0)
        for j in range(NKH):
            hw_j_group(1, 1, j)
        proj_half(1)

        if loop_cm is not None:
            loop_cm.__exit__(None, None, None)

    nc.compile()
    return nc


# ---------------- host-side preparation ----------------

def prep_shared(char_emb, conv_ws, conv_bs, hw_ws, hw_bs, proj_w, proj_b):
    out = {}
    # combined conv weight [120, 2048]
    wall = np.zeros((KCONV, TOTAL_F), dtype=np.float32)
    fbase = 0
    for (w, nf), cw, cb in zip(FILTERS, conv_ws, conv_bs):
        cw = np.asarray(cw, np.float32)  # [nf, 16, w]
        for dw in range(w):
            wall[16 * dw:16 * (dw + 1), fbase:fbase + nf] = cw[:, :, dw].T
        wall[112 + (w - 1), fbase:fbase + nf] = NEG if w > 1 else 0.0
        wall[119, fbase:fbase + nf] = np.asarray(cb, np.float32)
        fbase += nf
    out["wall"] = wall.astype(_BF)

    def repack(wm, no, dt=_BF, scale=1.0):
        wm = np.asarray(wm, np.float32) * scale
        kk = wm.shape[0] // 128
        return (wm.astype(dt).astype(np.float32).astype(dt)
                .reshape(kk, 128, no, 128)
                .transpose(2, 1, 0, 3).reshape(no, 128, kk * 128))

    F8NP = ml_dtypes.float8_e4m3
    out["hw0w"] = repack(hw_ws[0], 32, dt=F8NP, scale=WS)
    out["hw1w"] = repack(hw_ws[1], 32, dt=F8NP, scale=WS)
    out["prjw"] = repack(proj_w, NOP)

    hwb = np.zeros((128, 64), dtype=np.float32)
    for layer in range(2):
        hb = np.asarray(hw_bs[layer], np.float32)
        for j in range(16):
            hwb[:, layer * 32 + j] = hb[128 * j:128 * (j + 1)]
            hwb[:, layer * 32 + 16 + j] = hb[TOTAL_F + 128 * j:TOTAL_F + 128 * (j + 1)]
    out["hwb"] = hwb
    out["prjb"] = np.asarray(proj_b, np.float32).reshape(NOP, 128).T.copy()
    return out


# static mask pattern rows (host-merged into X_skew)
_PAT = None


def _pat_rows():
    global _PAT
    if _PAT is None:
        pat = np.zeros((8, COLS), dtype=_BF)
        pos = np.arange(COLS) % P50
        for j in range(7):
            pat[j] = (pos >= P50 - j).astype(_BF)
        pat[7] = 1.0
        _PAT = pat
    return _PAT


def prep_XS(ids_core, emb_bf):
    """ids_core [T, 50] -> host-skewed X_skew [120, COLS] bf16."""
    flat = np.zeros(COLS + 8, dtype=np.int64)
    flat[:T * P50] = ids_core.reshape(-1)
    G = emb_bf[flat]                       # [COLS+8, 16]
    xs = np.empty((KCONV, COLS), dtype=_BF)
    for g in range(7):
        xs[16 * g:16 * (g + 1), :] = G[g:g + COLS].T
    xs[112:120, :] = _pat_rows()
    return xs


_CACHED_NC = None


def _get_nc():
    global _CACHED_NC
    if _CACHED_NC is None:
        _CACHED_NC = build_program()
    return _CACHED_NC


def make_in_maps(inputs):
    ii = {k: np.asarray(v) for k, v in inputs.items()}
    conv_ws = [ii[f"conv_w{i}"] for i in range(7)]
    conv_bs = [ii[f"conv_b{i}"] for i in range(7)]
    shared = prep_shared(
        ii["char_emb"], conv_ws, conv_bs,
        [ii["hw_w0"], ii["hw_w1"]], [ii["hw_b0"], ii["hw_b1"]],
        ii["proj_w"], ii["proj_b"],
    )
    emb_bf = _bf(ii["char_emb"])  # [264, 16]
    ids = ii["input_ids"].reshape(-1, MAX_CHARS)
    in_maps = []
    for c in range(NCORES):
        m = dict(shared)
        m["XS"] = prep_XS(ids[c * T:(c + 1) * T], emb_bf)
        in_maps.append(m)
    return in_maps


def run(inputs, trace=False, **kw):
    in_maps = make_in_maps(inputs)
    res = run_bass_kernel_spmd(_get_nc(), in_maps, list(range(NCORES)),
                               trace=trace, **kw)
    outs = []
    for c in range(NCORES):
        o = np.asarray(res.results[c]["out"])  # [6, 128, T] fp32
        outs.append(o.reshape(HIDDEN, T).T)    # [T, 768]
    full = np.stack(outs, axis=0).reshape(B, S, HIDDEN).astype(np.float32)
    return full, res


def kernel(**inputs):
    return run(inputs)[0]


if __name__ == "__main__":
    build_program()
    print("build ok")


# revision 12
# speedup vs baseline: 1.0758x; 1.0758x over previous
"""Trainium2 Bass kernel v2 for CharacterBERT CharCNN.

v2 design:
  - Embedding gather AND skew done on HOST: device receives a ready
    x_skew=[120, 25600] bf16 (112 shifted-embedding rows + 7 mask rows +
    ones row), uploaded in column slices so conv starts almost immediately.
  - Conv = one matmul per 128-filter chunk against combined width-padded
    weights W_all [120, 2048] (masks inject -30000; bias via ones row).
  - Maxpool+relu via split ACT/DVE pooling: ACT-path chunks are drained
    PSUM->SBUF bf16 by the scalar engine (free Relu) in pos-major layout,
    then reduced by a 6-level DVE tensor_tensor max tree (2x mode);
    DVE-path chunks use one 1x tensor_reduce per PSUM tile.  ACT-path and
    DVE-path chunks are processed in PAIRS with tile-level interleaving and
    a 3-slot PSUM ring so both drain engines run concurrently.
  - Highway/proj matmuls per 256-token half; half-0 highway interleaves
    with half-1 conv+drain in PE program order (PE executes in-order).
"""

import numpy as np
import ml_dtypes
from contextlib import ExitStack

import concourse.bass as bass
import concourse.mybir as mybir
import concourse.tile as tile
from concourse import bacc
from concourse.bass_utils import run_bass_kernel_spmd

BF16 = mybir.dt.bfloat16
F32 = mybir.dt.float32
AF = mybir.ActivationFunctionType
ALU = mybir.AluOpType

# problem geometry (hardcoded)
B, S, MAX_CHARS = 8, 512, 50
EMB = 16
VOCAB = 264
TOTAL_F = 2048
HIDDEN = 768
FILTERS = [(1, 32), (2, 32), (3, 64), (4, 128), (5, 256), (6, 512), (7, 1024)]
NCORES = 8

T = 512                 # tokens per core
HALF = 256
P50 = MAX_CHARS
COLS = T * P50          # 25600
KCONV = 120             # 112 patch rows + 7 mask rows + 1 bias row
NEG = -30000.0
NKF = TOTAL_F // 128    # 16 filter chunks
NKH = TOTAL_F // 128    # 16 contraction chunks
NOP = HIDDEN // 128     # 6 proj chunks
NSLICE = 8              # x_skew upload column slices
WS = 64.0               # fp8 highway-weight scale

# valid conv output positions per 128-filter chunk (width-trimmed):
# chunk 0 = w1+w2+w3 (needs 50, masked), 1 = w4, 2-3 = w5, 4-7 = w6, 8-15 = w7
PCOL = [50, 47, 46, 46, 45, 45, 45, 45] + [44] * 8

# chunk processing order per half: (ACT-path chunk, DVE-path chunk or None)
PAIR_ORDER = [(0, 2), (1, None), (3, 5), (4, None), (6, 8), (7, None),
              (9, 11), (10, None), (12, 14), (13, None), (15, None)]
DVE_CHUNKS = frozenset(d for _, d in PAIR_ORDER if d is not None)

_BF = ml_dtypes.bfloat16


def _bf(x):
    return np.asarray(x, dtype=np.float32).astype(_BF)


def _tiles_of(n, step):
    out, t0 = [], 0
    while t0 < n:
        out.append((t0, min(step, n - t0)))
        t0 += step
    return out


def _tree_plan(p):
    """Overlapped pairwise-max splits: P -> a=ceil(P/2) via max(x[0:a], x[P-a:P])."""
    plan = []
    while p > 1:
        a = (p + 1) // 2
        plan.append((p, a))
        p = a
    return plan


def build_program(ExitStackCls=ExitStack, loop_n=1, variant=""):
    """variant: comma-joined bisection flags:
    'hwbf16'  - highway weights bf16 instead of fp8 (needs bf16 in_maps)
    'alldve'  - drain every chunk via DVE tensor_reduce (no ACT path/tree)
    'noconv'  - skip conv matmuls (drains read stale psum; timing only)
    'nohw'    - skip highway+proj (timing only)
    """
    nc = bacc.Bacc("TRN2", target_bir_lowering=False, debug=False)
    vset = set(v for v in variant.split(",") if v)

    F8 = BF16 if "hwbf16" in vset else mybir.dt.float8e4
    d_XS = nc.dram_tensor("XS", [KCONV, COLS], BF16, kind="ExternalInput").ap()
    d_wall = nc.dram_tensor("wall", [KCONV, TOTAL_F], BF16, kind="ExternalInput").ap()
    d_hw0w = nc.dram_tensor("hw0w", [32, 128, TOTAL_F], F8, kind="ExternalInput").ap()
    d_hw1w = nc.dram_tensor("hw1w", [32, 128, TOTAL_F], F8, kind="ExternalInput").ap()
    d_prjw = nc.dram_tensor("prjw", [NOP, 128, TOTAL_F], BF16, kind="ExternalInput").ap()
    d_hwb = nc.dram_tensor("hwb", [128, 64], F32, kind="ExternalInput").ap()
    d_prjb = nc.dram_tensor("prjb", [128, NOP], F32, kind="ExternalInput").ap()
    d_out = nc.dram_tensor("out", [NOP, 128, T], F32, kind="ExternalOutput").ap()
    d_hw_w = [d_hw0w, d_hw1w]

    with tile.TileContext(nc) as tc, ExitStackCls() as ctx:
        const = ctx.enter_context(tc.tile_pool(name="const", bufs=1))
        xsk_p = ctx.enter_context(tc.tile_pool(name="xsk", bufs=1))
        dr_p = ctx.enter_context(tc.tile_pool(name="dr", bufs=2))
        t_p = ctx.enter_context(tc.tile_pool(name="tmaj", bufs=1))
        hww_p = ctx.enter_context(tc.tile_pool(name="hww", bufs=3))
        hwt_p = ctx.enter_context(tc.tile_pool(name="hwt", bufs=2))
        out_p = ctx.enter_context(tc.tile_pool(name="outp", bufs=2))
        convps = ctx.enter_context(tc.tile_pool(name="convps", bufs=3, space="PSUM"))
        hwps = ctx.enter_context(tc.tile_pool(name="hwps", bufs=2, space="PSUM"))

        # ---- constants + skewed input (host-built, column-sliced upload) ----
        wall_t = const.tile([KCONV, TOTAL_F], BF16)
        nc.sync.dma_start(wall_t[:], d_wall[:])
        hwb_t = const.tile([128, 64], F32)
        nc.sync.dma_start(hwb_t[:], d_hwb[:])
        prjb_t = const.tile([128, NOP], F32)
        nc.sync.dma_start(prjb_t[:], d_prjb[:])

        x_skew = xsk_p.tile([KCONV, COLS], BF16)
        for (s0, ns) in _tiles_of(COLS, COLS // NSLICE):
            nc.sync.dma_start(out=x_skew[:, s0:s0 + ns],
                              in_=d_XS[:, s0:s0 + ns])

        # per-half t tiles, 3 generations (t0 drained, t1, t2)
        t_tiles = {}
        for gen in range(3):
            for h in range(2):
                t_tiles[(gen, h)] = t_p.tile([128, NKF, HALF], BF16,
                                             tag=f"t{gen}{h}", name=f"t{gen}{h}")

        loop_cm = tc.For_i(0, loop_n) if loop_n > 1 else None
        if loop_cm is not None:
            loop_cm.__enter__()

        x3 = x_skew[:].rearrange("p (t q) -> p t q", q=P50)

        def tok_groups(ck):
            """Drain-tile token groups per half for chunk ck.

            Each tile holds up to 2 matmul groups of n=floor(512/P) tokens,
            each MM landing in its own PSUM bank (offsets 0 and 512)."""
            p = PCOL[ck]
            n = 512 // p
            tiles, t0 = [], 0
            while t0 < HALF:
                g1 = min(n, HALF - t0)
                g2 = min(n, HALF - t0 - g1)
                tiles.append((t0, g1, g2))
                t0 += g1 + g2
            return tiles

        def conv_tile(h, ck, tt0, g1, g2, pos_major=False):
            """Conv matmuls for one drain tile; returns the psum tile.

            pos_major streams the moving operand in (pos, tok) order so the
            PSUM layout is [pos, tok] - the ACT drain then writes the
            pos-major dr buffer contiguously."""
            p = PCOL[ck]
            ps = convps.tile([128, 1024], F32, tag="cps", name="cps")
            base_t = h * HALF + tt0
            for k, ng in enumerate((g1, g2)):
                if ng:
                    rhs = x3[:, base_t + k * g1:base_t + k * g1 + ng, 0:p]
                    if pos_major:
                        rhs = rhs.transpose([0, 2, 1])
                    nc.tensor.matmul(
                        ps[:, 512 * k:512 * k + ng * p],
                        lhsT=wall_t[:, 128 * ck:128 * (ck + 1)],
                        rhs=rhs,
                        start=True, stop=True,
                    )
            return ps

        def drain_act(ps, dr, ck, tt0, g1, g2):
            p = PCOL[ck]
            if g1 == g2:
                nc.scalar.activation(
                    out=dr[:, 0:p, tt0:tt0 + 2 * g1].transpose([0, 2, 1]),
                    in_=ps[:].rearrange("p (s c) -> p s c", s=2)[:, :, 0:g1 * p],
                    func=AF.Relu,
                )
            else:
                for k, (o, ng) in enumerate(((0, g1), (g1, g2))):
                    if ng:
                        nc.scalar.activation(
                            out=dr[:, 0:p, tt0 + o:tt0 + o + ng]
                                .transpose([0, 2, 1]),
                            in_=ps[:, 512 * k:512 * k + ng * p],
                            func=AF.Relu,
                        )

        def drain_act_pm(ps, dr, ck, tt0, g1, g2):
            """pos-major psum -> contiguous ACT writes into dr."""
            p = PCOL[ck]
            if g1 == g2:
                nc.scalar.activation(
                    out=dr[:, 0:p, tt0:tt0 + 2 * g1]
                        .rearrange("c q (s t) -> c s q t", s=2),
                    in_=ps[:].rearrange("p (s c) -> p s c", s=2)[:, :, 0:g1 * p],
                    func=AF.Relu,
                )
            else:
                for k, (o, ng) in enumerate(((0, g1), (g1, g2))):
                    if ng:
                        nc.scalar.activation(
                            out=dr[:, 0:p, tt0 + o:tt0 + o + ng],
                            in_=ps[:, 512 * k:512 * k + ng * p],
                            func=AF.Relu,
                        )

        def drain_dve(ps, t0t, ck, tt0, g1, g2):
            p = PCOL[ck]
            if g1 == g2:
                nc.vector.tensor_reduce(
                    out=t0t[:, ck, tt0:tt0 + 2 * g1],
                    in_=ps[:].rearrange("p (s c) -> p s c", s=2)
                        [:, :, 0:g1 * p].rearrange("p s (t q) -> p s t q", q=p),
                    axis=mybir.AxisListType.X,
                    op=ALU.max,
                )
            else:
                for k, (o, ng) in enumerate(((0, g1), (g1, g2))):
                    if ng:
                        nc.vector.tensor_reduce(
                            out=t0t[:, ck, tt0 + o:tt0 + o + ng],
                            in_=ps[:, 512 * k:512 * k + ng * p]
                                .rearrange("p (t q) -> p t q", q=p),
                            axis=mybir.AxisListType.X,
                            op=ALU.max,
                        )

        def tree(dr, t0t, ck):
            # pairwise-max tree; ping-pong row regions A=[50,75) B=[0,50)
            dst_seq = [50, 0, 25, 0, 25]
            cur_base = 0
            plan = _tree_plan(PCOL[ck])
            for li, (p, a) in enumerate(plan):
                in0 = dr[:, cur_base:cur_base + a, :]
                in1 = dr[:, cur_base + p - a:cur_base + p, :]
                if li == len(plan) - 1:
                    nc.vector.tensor_max(t0t[:, ck, :], in0[:, 0, :],
                                         in1[:, 0, :])
                else:
                    dst = dst_seq[li]
                    nc.vector.tensor_max(dr[:, dst:dst + a, :], in0, in1)
                    cur_base = dst

        def conv_drain_group(h, ack, dck):
            """One ACT-path chunk (+ optionally one DVE-path chunk),
            tile-interleaved so ACT and DVE drain concurrently."""
            t0t = t_tiles[(0, h)]
            if "alldve" in vset:
                for ck in ([ack] if dck is None else [ack, dck]):
                    for tg in tok_groups(ck):
                        ps = conv_tile(h, ck, *tg)
                        drain_dve(ps, t0t, ck, *tg)
                    nc.vector.tensor_scalar_max(t0t[:, ck, :],
                                                t0t[:, ck, :], 0.0)
                return
            dr = dr_p.tile([128, 75, HALF], BF16, tag="dr", name="dr")
            pm = "posmajor" in vset
            da = drain_act_pm if pm else drain_act
            ta = tok_groups(ack)
            td = tok_groups(dck) if dck is not None else []
            for i in range(max(len(ta), len(td))):
                ps_a = (conv_tile(h, ack, *ta[i], pos_major=pm)
                        if i < len(ta) else None)
                ps_d = conv_tile(h, dck, *td[i]) if i < len(td) else None
                if ps_a is not None:
                    da(ps_a, dr, ack, *ta[i])
                if ps_d is not None:
                    drain_dve(ps_d, t0t, dck, *td[i])
            tree(dr, t0t, ack)
            if dck is not None:
                nc.vector.tensor_scalar_max(t0t[:, dck, :], t0t[:, dck, :], 0.0)

        def hw_j_group(layer, h, j):
            t_in = t_tiles[(layer, h)]
            t_out = t_tiles[(layer + 1, h)]
            w2 = hww_p.tile([128, 2 * TOTAL_F], F8, tag="w", name="w2")
            nc.sync.dma_start(w2[:, 0:TOTAL_F], d_hw_w[layer][j, :, :])
            nc.sync.dma_start(w2[:, TOTAL_F:2 * TOTAL_F],
                              d_hw_w[layer][j + 16, :, :])
            b_nl = hwb_t[:, layer * 32 + j:layer * 32 + j + 1]
            b_g = hwb_t[:, layer * 32 + 16 + j:layer * 32 + 16 + j + 1]
            ps = hwps.tile([128, 512], F32, tag="hw", name="hwps")
            for kc in range(NKH):
                nc.tensor.matmul(
                    ps[:, 0:HALF],
                    lhsT=w2[:, 128 * kc:128 * (kc + 1)],
                    rhs=t_in[:, kc, :],
                    start=(kc == 0), stop=(kc == NKH - 1),
                )
            for kc in range(NKH):
                nc.tensor.matmul(
                    ps[:, HALF:2 * HALF],
                    lhsT=w2[:, TOTAL_F + 128 * kc:TOTAL_F + 128 * (kc + 1)],
                    rhs=t_in[:, kc, :],
                    start=(kc == 0), stop=(kc == NKH - 1),
                )
            sg = hwt_p.tile([128, HALF], BF16, tag="sg", name="sg")
            nc.scalar.activation(sg[:], ps[:, HALF:2 * HALF], AF.Sigmoid,
                                 bias=b_g, scale=1.0 / WS)
            rl = hwt_p.tile([128, HALF], BF16, tag="rl", name="rl")
            nc.scalar.activation(rl[:], ps[:, 0:HALF], AF.Relu, bias=b_nl,
                                 scale=1.0 / WS)
            dd = hwt_p.tile([128, HALF], BF16, tag="dd", name="dd")
            nc.vector.tensor_sub(dd[:], t_in[:, j, :], rl[:])
            ee = hwt_p.tile([128, HALF], BF16, tag="ee", name="ee")
            nc.vector.tensor_mul(ee[:], sg[:], dd[:])
            nc.vector.tensor_add(t_out[:, j, :], ee[:], rl[:])

        def proj_half(h):
            t_in = t_tiles[(2, h)]
            for o in range(NOP):
                w2 = hww_p.tile([128, 2 * TOTAL_F], BF16, tag="w", name="w2")
                nc.sync.dma_start(w2[:, 0:TOTAL_F], d_prjw[o, :, :])
                ps = hwps.tile([128, 512], F32, tag="hw", name="hwps")
                for kc in range(NKH):
                    nc.tensor.matmul(
                        ps[:, 0:HALF],
                        lhsT=w2[:, 128 * kc:128 * (kc + 1)],
                        rhs=t_in[:, kc, :],
                        start=(kc == 0), stop=(kc == NKH - 1),
                    )
                ot = out_p.tile([128, HALF], F32, tag="ot", name="ot")
                nc.scalar.activation(ot[:], ps[:, 0:HALF], AF.Identity,
                                     bias=prjb_t[:, o:o + 1])
                nc.sync.dma_start(out=d_out[o, :, h * HALF:(h + 1) * HALF],
                                  in_=ot[:])

        # ---- emission order (PE in-order; this shapes the overlap) ----
        nohw = "nohw" in vset
        for (ack, dck) in PAIR_ORDER:
            conv_drain_group(0, ack, dck)

        # interleave conv+drain(half 1) with highway L1+L2(half 0):
        # 11 conv groups vs 32 hw j-groups -> conv, hw, hw, hw, conv, ...
        hw0_units = [(0, j) for j in range(NKH)] + [(1, j) for j in range(NKH)]
        ci = hi = 0
        while ci < len(PAIR_ORDER) or hi < len(hw0_units):
            if ci < len(PAIR_ORDER) and (hi >= len(hw0_units)
                                         or hi * len(PAIR_ORDER) >= ci * len(hw0_units)):
                ack, dck = PAIR_ORDER[ci]
                conv_drain_group(1, ack, dck)
                ci += 1
            else:
                layer, j = hw0_units[hi]
                if not nohw:
                    hw_j_group(layer, 0, j)
                hi += 1

        if not nohw:
            for j in range(NKH):
                hw_j_group(0, 1, j)
        proj_half(0)
        if not nohw:
            for j in range(NKH):
                hw_j_group(1, 1, j)
        proj_half(1)

        if loop_cm is not None:
            loop_cm.__exit__(None, None, None)

    nc.compile()
    return nc


# ---------------- host-side preparation ----------------

def prep_shared(char_emb, conv_ws, conv_bs, hw_ws, hw_bs, proj_w, proj_b, hw_fp8=True):
    out = {}
    # combined conv weight [120, 2048]
    wall = np.zeros((KCONV, TOTAL_F), dtype=np.float32)
    fbase = 0
    for (w, nf), cw, cb in zip(FILTERS, conv_ws, conv_bs):
        cw = np.asarray(cw, np.float32)  # [nf, 16, w]
        for dw in range(w):
            wall[16 * dw:16 * (dw + 1), fbase:fbase + nf] = cw[:, :, dw].T
        wall[112 + (w - 1), fbase:fbase + nf] = NEG if w > 1 else 0.0
        wall[119, fbase:fbase + nf] = np.asarray(cb, np.float32)
        fbase += nf
    out["wall"] = wall.astype(_BF)

    def repack(wm, no, dt=_BF, scale=1.0):
        wm = np.asarray(wm, np.float32) * scale
        kk = wm.shape[0] // 128
        return (wm.astype(dt).astype(np.float32).astype(dt)
                .reshape(kk, 128, no, 128)
                .transpose(2, 1, 0, 3).reshape(no, 128, kk * 128))

    F8NP = ml_dtypes.float8_e4m3
    if hw_fp8:
        out["hw0w"] = repack(hw_ws[0], 32, dt=F8NP, scale=WS)
        out["hw1w"] = repack(hw_ws[1], 32, dt=F8NP, scale=WS)
    else:
        out["hw0w"] = repack(hw_ws[0], 32, scale=WS)
        out["hw1w"] = repack(hw_ws[1], 32, scale=WS)
    out["prjw"] = repack(proj_w, NOP)

    hwb = np.zeros((128, 64), dtype=np.float32)
    for layer in range(2):
        hb = np.asarray(hw_bs[layer], np.float32)
        for j in range(16):
            hwb[:, layer * 32 + j] = hb[128 * j:128 * (j + 1)]
            hwb[:, layer * 32 + 16 + j] = hb[TOTAL_F + 128 * j:TOTAL_F + 128 * (j + 1)]
    out["hwb"] = hwb
    out["prjb"] = np.asarray(proj_b, np.float32).reshape(NOP, 128).T.copy()
    return out


# static mask pattern rows (host-merged into X_skew)
_PAT = None


def _pat_rows():
    global _PAT
    if _PAT is None:
        pat = np.zeros((8, COLS), dtype=_BF)
        pos = np.arange(COLS) % P50
        for j in range(7):
            pat[j] = (pos >= P50 - j).astype(_BF)
        pat[7] = 1.0
        _PAT = pat
    return _PAT


def prep_XS(ids_core, emb_bf):
    """ids_core [T, 50] -> host-skewed X_skew [120, COLS] bf16."""
    flat = np.zeros(COLS + 8, dtype=np.int64)
    flat[:T * P50] = ids_core.reshape(-1)
    G = emb_bf[flat]                       # [COLS+8, 16]
    xs = np.empty((KCONV, COLS), dtype=_BF)
    for g in range(7):
        xs[16 * g:16 * (g + 1), :] = G[g:g + COLS].T
    xs[112:120, :] = _pat_rows()
    return xs


_CACHED_NC = None


def _get_nc():
    global _CACHED_NC
    if _CACHED_NC is None:
        _CACHED_NC = build_program()
    return _CACHED_NC


def make_in_maps(inputs, hw_fp8=True):
    ii = {k: np.asarray(v) for k, v in inputs.items()}
    conv_ws = [ii[f"conv_w{i}"] for i in range(7)]
    conv_bs = [ii[f"conv_b{i}"] for i in range(7)]
    shared = prep_shared(
        ii["char_emb"], conv_ws, conv_bs,
        [ii["hw_w0"], ii["hw_w1"]], [ii["hw_b0"], ii["hw_b1"]],
        ii["proj_w"], ii["proj_b"], hw_fp8=hw_fp8,
    )
    emb_bf = _bf(ii["char_emb"])  # [264, 16]
    ids = ii["input_ids"].reshape(-1, MAX_CHARS)
    in_maps = []
    for c in range(NCORES):
        m = dict(shared)
        m["XS"] = prep_XS(ids[c * T:(c + 1) * T], emb_bf)
        in_maps.append(m)
    return in_maps


def run(inputs, trace=False, **kw):
    in_maps = make_in_maps(inputs)
    res = run_bass_kernel_spmd(_get_nc(), in_maps, list(range(NCORES)),
                               trace=trace, **kw)
    outs = []
    for c in range(NCORES):
        o = np.asarray(res.results[c]["out"])  # [6, 128, T] fp32
        outs.append(o.reshape(HIDDEN, T).T)    # [T, 768]
    full = np.stack(outs, axis=0).reshape(B, S, HIDDEN).astype(np.float32)
    return full, res


def kernel(**inputs):
    return run(inputs)[0]


if __name__ == "__main__":
    build_program()
    print("build ok")
